# revision 21
# baseline (speedup 1.0000x reference)
"""LoRA-linear Trainium2 Bass kernel (bf16 in/out, chunk-streamed, pipelined).

Computes, for T adapters: out[t] = x @ W.T + (x @ A_t.T) @ B_t.T + bias
Output: [T, B, S, Dout] float32 (stored bf16 on-device, widened on host).

Sharding: data-parallel over tokens across 8 NeuronCores (2048 tokens/core);
W/bias/selected-LoRA replicated. All matmul inputs are bf16 (host cast);
accumulation is fp32 in PSUM; the output is written to HBM as bf16 (16 MB
per core instead of 32), far inside the 2e-2 absmax-relative gate.

Lessons from the NTFF traces baked in here:
 * Every load/store is a single plain DMA with >=2KB contiguous
   per-partition runs (host pre-packs all layouts). Small or strided
   patterns cost 100s of descriptors; descriptor-gen serializes on the
   issuing engine's queue (~0.7us per 128-descriptor DMA) and tiny runs
   drain far below line rate. An early version spent 90us issuing 128
   per-(c,m,t) stores; this one issues 32 per-(c,m) stores of
   [128, T*512] (4KB/partition contiguous both sides).
 * Warm-up matmuls read a GpSimd-memset tile, not a DMA'd tile, so the
   HAM clock-gate ramp (needs ~3.4us of PE activity) runs during the DMA
   prologue instead of after it.
 * Loads are ordered by first use and sliced to stay just ahead of the
   consumer: x-chunk0 in two k-halves (phase1 k0-3 starts on the first),
   then W m-tile 0 alone, m1-3, then B/bias, W m4-7, x chunks 1-3.
 * DVE tensor_tensor with a PSUM operand runs at 1x; all-bf16 SBUF adds
   run at 2x. Per (c,m): deltas t0/t1 land in one 2-bank PSUM tile that
   is added in a single 1024-wide op (base broadcast via a 0-stride AP),
   t2/t3 land in another, which ScalarE copies to bf16 SBUF (one fused
   ACTIVATE) before a 1024-wide 2x DVE add. DVE (~61us) and ScalarE
   (~60us) both sit under the PE's ~77us.

Per-core schedule, chunk-major over 4 token-chunks of 512 tokens:
  phase1(c) lowT[32t+j, tok] = sum_d A_t[j,d] x[tok,d]  (8 k-matmuls)
  base(c,m) W[m-tile] @ x_c.T -> PSUM (8 k-matmuls, 216ns cadence);
            ScalarE evacuates with bias folded in, bf16
  delta     per t: 4 row-group matmuls at tile_position (32t,0),
            concurrent in the PE array (~0.4us for all four)
  add/store DVE adds write bf16 slices of od[128, T*512]; one 512KB store
Deltas for (c, m-1) are emitted after base (c, m) so the PE never waits
on DVE/ScalarE; phase1(c+1) is slotted mid-chunk, off the critical path.
"""

import sys

if "/opt/trn_rl_repo" not in sys.path:
    sys.path.insert(0, "/opt/trn_rl_repo")

from contextlib import ExitStack

import ml_dtypes
import numpy as np

import concourse.bacc as bacc
import concourse.bass as bass
import concourse.mybir as mybir
import concourse.tile as tile
from concourse import bass_utils

# Problem constants (hardcoded per spec).
B, S, DIN, DOUT, R, NL, T = 4, 4096, 1024, 1024, 16, 8, 4
NCORES = 8
NTOK = B * S                 # 16384
CTOK = NTOK // NCORES        # 2048 tokens per core
KT = DIN // 128              # 8 k-tiles
MT = DOUT // 128             # 8 dout-tiles
CH = 4                       # token chunks per core
CW = CTOK // CH              # 512 tokens per chunk

# Warm-up must keep the PE continuously busy from engine-start (~7.5us)
# until phase1's inputs land (~13us): any idle gap resets the HAM
# activity window and the main loop opens at 1.2 GHz instead of 2.4.
WARM1 = 55

F32 = mybir.dt.float32
BF16 = mybir.dt.bfloat16
NPBF16 = ml_dtypes.bfloat16


def _build_program():
    nc = bacc.Bacc("TRN2", target_bir_lowering=False, debug=False,
                   num_devices=NCORES)

    # All DRAM layouts are pre-packed on host so every DMA is a plain
    # contiguous [128, n] transfer.
    xc = nc.dram_tensor("xc", [CH, 128, KT * CW], BF16, kind="ExternalInput").ap()
    wt = nc.dram_tensor("wt", [128, MT * KT * 128], BF16, kind="ExternalInput").ap()
    atp = nc.dram_tensor("atp", [128, KT * 128], BF16, kind="ExternalInput").ap()
    btp = nc.dram_tensor("btp", [128, DOUT], BF16, kind="ExternalInput").ap()
    biasc = nc.dram_tensor("biasc", [128, MT], F32, kind="ExternalInput").ap()
    out = nc.dram_tensor("out", [CH, MT, 128, T * CW], BF16,
                         kind="ExternalOutput").ap()

    with tile.TileContext(nc) as tc, ExitStack() as ctx:
        const = ctx.enter_context(tc.tile_pool(name="const", bufs=1))
        lw_pool = ctx.enter_context(tc.tile_pool(name="lw", bufs=2))
        bsb_pool = ctx.enter_context(tc.tile_pool(name="bsb", bufs=3))
        ds_pool = ctx.enter_context(tc.tile_pool(name="ds", bufs=3))
        od_pool = ctx.enter_context(tc.tile_pool(name="od", bufs=6))
        bp_ps = ctx.enter_context(tc.tile_pool(name="bp_ps", bufs=2, space="PSUM"))
        ph_ps = ctx.enter_context(tc.tile_pool(name="ph_ps", bufs=2, space="PSUM"))
        dps_ps = ctx.enter_context(tc.tile_pool(name="dps_ps", bufs=1, space="PSUM"))
        dpd_ps = ctx.enter_context(tc.tile_pool(name="dpd_ps", bufs=1, space="PSUM"))

        # Warm-up source: memset by GpSimd (~6us mark), no DMA dependency.
        wsrc = const.tile([128, 128], BF16, tag="wsrc")
        nc.gpsimd.memset(wsrc[:], 0.0)

        # Only A on the scalar HWDGE ring (phase1 gate); B/bias ride the
        # sync ring behind the early W slices so x-chunk0 drains at a
        # larger SDMA share.
        at_all = const.tile([128, KT * 128], BF16, tag="at")
        nc.scalar.dma_start(at_all[:], atp[:, :])
        bt_s = const.tile([128, DOUT], BF16, tag="bt")
        bias_s = const.tile([128, MT], F32, tag="bias")

        # Big loads on the sync ring, in consumption order: x chunk 0
        # (phase1 needs only x), W in two m-halves, then chunks 1-3.
        xc_t = []
        for c in range(CH):
            xc_t.append(const.tile([128, KT * CW], BF16, tag=f"xc{c}",
                                   name=f"xc{c}"))
        wt_all = const.tile([128, MT * KT * 128], BF16, tag="wt")

        # x chunk 0 in two k-halves (phase1 k0-3 can start on the first),
        # then W in m-slices sized to stay ahead of the base m-loop.
        hx = KT * CW // 2
        nc.sync.dma_start(xc_t[0][:, 0:hx], xc[0][:, 0:hx])
        nc.sync.dma_start(xc_t[0][:, hx:2 * hx], xc[0][:, hx:2 * hx])
        msz = KT * 128
        nc.sync.dma_start(wt_all[:, 0:msz], wt[:, 0:msz])
        nc.sync.dma_start(wt_all[:, msz:4 * msz], wt[:, msz:4 * msz])
        nc.sync.dma_start(bt_s[:], btp[:, :])
        nc.sync.dma_start(bias_s[:], biasc[:, :])
        nc.sync.dma_start(wt_all[:, 4 * msz:MT * msz], wt[:, 4 * msz:MT * msz])
        for c in range(1, CH):
            nc.sync.dma_start(xc_t[c][:], xc[c])

        lwt = {}

        def emit_phase1(c):
            ph = ph_ps.tile([128, CW], F32, tag="ph", name=f"ph{c}")
            for k in range(KT):
                nc.tensor.matmul(
                    ph[:],
                    at_all[:, bass.ts(k, 128)],
                    xc_t[c][:, bass.ts(k, CW)],
                    start=(k == 0), stop=(k == KT - 1),
                )
            t_ = lw_pool.tile([128, CW], BF16, tag="lw", name=f"lw{c}")
            nc.scalar.copy(t_[:], ph[:])
            lwt[c] = t_

        # Warm-up: the HAM clock gate needs ~3.4us of sustained PE activity
        # to unthrottle 1.2 -> 2.4 GHz; run it on the memset tile while the
        # input DMAs stream.
        warm = ph_ps.tile([128, CW], F32, tag="ph", name="warm")
        for _ in range(WARM1):
            nc.tensor.matmul(warm[:, 0:128], wsrc[:], wsrc[:],
                             start=True, stop=True)
        emit_phase1(0)

        def emit_base_mms(c, m):
            bp = bp_ps.tile([128, CW], F32, tag="bp", name=f"bp{c}_{m}")
            for k in range(KT):
                nc.tensor.matmul(
                    bp[:],
                    wt_all[:, m * (KT * 128) + k * 128:
                           m * (KT * 128) + (k + 1) * 128],
                    xc_t[c][:, bass.ts(k, CW)],
                    start=(k == 0), stop=(k == KT - 1),
                )
            return bp

        def emit_evac(c, m, bp):
            bsb = bsb_pool.tile([128, CW], BF16, tag="bsb", name=f"bsb{c}_{m}")
            nc.scalar.activation(
                bsb[:], bp[:],
                mybir.ActivationFunctionType.Identity,
                bias=bias_s[:, m:m + 1],
            )
            return bsb

        def emit_delta(c, m, bsb, last=False):
            d01 = dps_ps.tile([128, 2 * CW], F32, tag="dps", name=f"dp{c}_{m}_01")
            d23 = dpd_ps.tile([128, 2 * CW], F32, tag="dpd", name=f"dp{c}_{m}_23")
            outs = [d01[:, 0:CW], d01[:, CW:2 * CW],
                    d23[:, 0:CW], d23[:, CW:2 * CW]]
            for t in range(T):
                nc.tensor.matmul(
                    outs[t],
                    bt_s[32 * t:32 * t + R, bass.ts(m, 128)],
                    lwt[c][32 * t:32 * t + R, :],
                    start=True, stop=True,
                    tile_position=(32 * t, 0),
                )
            # Fused ScalarE evacuation of t2/t3 ahead of the base evac in the
            # ACT queue so the (single-buffered) d23 bank frees early.
            ds = ds_pool.tile([128, 2 * CW], BF16, tag="ds", name=f"ds{c}_{m}")
            nc.scalar.copy(ds[:], d23[:])
            od = od_pool.tile([128, T * CW], BF16, tag="od", name=f"od{c}_{m}")
            # Fused pair-adds: base broadcast along a 0-stride outer dim.
            bsb2 = bsb[:].rearrange("p (o w) -> p o w", o=1).broadcast_to(
                [128, 2, CW])
            nc.vector.tensor_add(
                od[:, 0:2 * CW].rearrange("p (o w) -> p o w", o=2),
                bsb2, d01[:].rearrange("p (o w) -> p o w", o=2))
            if last:
                # Final tile: two half-stores on different HWDGE rings so
                # drain + HBM write receipt overlap at the very end.
                nc.scalar.dma_start(out[c, m, :, 0:2 * CW], od[:, 0:2 * CW])
            nc.vector.tensor_add(
                od[:, 2 * CW:4 * CW].rearrange("p (o w) -> p o w", o=2),
                bsb2, ds[:].rearrange("p (o w) -> p o w", o=2))
            if last:
                nc.sync.dma_start(out[c, m, :, 2 * CW:4 * CW],
                                  od[:, 2 * CW:4 * CW])
            else:
                nc.sync.dma_start(out[c, m, :, :], od[:])

        prev = None
        for c in range(CH):
            for m in range(MT):
                if (c, m) == (CH - 1, MT - 1) and prev is not None:
                    # Break the software pipeline for the final tile: its
                    # predecessor's deltas/adds run concurrently with this
                    # base (deltas have no base dependency), so the tail
                    # chain after the last matmul shortens by ~1.5us.
                    emit_delta(*prev)
                    prev = None
                bp = emit_base_mms(c, m)
                if m == 4 and c + 1 < CH:
                    emit_phase1(c + 1)
                if prev is not None:
                    emit_delta(*prev)
                bsb = emit_evac(c, m, bp)
                prev = (c, m, bsb)
        emit_delta(*prev, last=True)

    nc.compile()
    return nc


_NC = None


def _get_program():
    global _NC
    if _NC is None:
        _NC = _build_program()
    return _NC


def kernel(**inputs):
    x = np.ascontiguousarray(np.asarray(inputs["x"], dtype=np.float32))
    W = np.asarray(inputs["W"], dtype=np.float32)
    bias_v = np.asarray(inputs["bias"], dtype=np.float32)
    lora_A = np.asarray(inputs["lora_A"], dtype=np.float32)
    lora_B = np.asarray(inputs["lora_B"], dtype=np.float32)
    tuner_index = np.asarray(inputs["tuner_index"]).astype(np.int64)

    assert x.shape == (B, S, DIN) and W.shape == (DOUT, DIN)
    assert tuner_index.shape == (T,)

    A_sel = lora_A[tuner_index]          # [T, R, Din]
    B_sel = lora_B[tuner_index]          # [T, Dout, R]

    toks = x.reshape(NTOK, DIN)
    # wt[p, m, k, n] = W[m*128+n, k*128+p]
    wt = np.ascontiguousarray(
        W.reshape(MT, 128, KT, 128).transpose(3, 0, 2, 1)
    ).astype(NPBF16).reshape(128, MT * KT * 128)
    # atp_flat[d, 32t+j] = A_sel[t, j, d]; then [p, k, j] = [k*128+p, j]
    atp_flat = np.zeros((DIN, 128), np.float32)
    atp_flat.reshape(DIN, T, 32)[:, :, :R] = A_sel.transpose(2, 0, 1)
    atp = np.ascontiguousarray(
        atp_flat.reshape(KT, 128, 128).transpose(1, 0, 2)
    ).astype(NPBF16).reshape(128, KT * 128)
    btp = np.zeros((128, DOUT), NPBF16)
    btp.reshape(T, 32, DOUT)[:, :R, :] = B_sel.transpose(0, 2, 1).astype(NPBF16)
    biasc = np.ascontiguousarray(bias_v.reshape(MT, 128).T)   # [128, MT]

    in_maps = []
    for c in range(NCORES):
        xcore = toks[c * CTOK:(c + 1) * CTOK]            # [2048, 1024]
        # xh[ch, p, k, w] = x[ch*512+w, k*128+p]
        xch = np.ascontiguousarray(
            xcore.reshape(CH, CW, KT, 128).transpose(0, 3, 2, 1)
        ).astype(NPBF16).reshape(CH, 128, KT * CW)
        in_maps.append({
            "xc": xch,
            "wt": wt,
            "atp": atp,
            "btp": btp,
            "biasc": biasc,
        })

    nc = _get_program()
    res = bass_utils.run_bass_kernel_spmd(nc, in_maps, core_ids=list(range(NCORES)))

    full = np.empty((T, NTOK, DOUT), np.float32)
    for c in range(NCORES):
        o = np.asarray(res.results[c]["out"])   # [CH, MT, 128, T*CW] bf16
        # o[ch, m, p, t, w] -> [t, ch*CW + w, m*128 + p]
        oc = o.reshape(CH, MT, 128, T, CW).transpose(3, 0, 4, 1, 2) \
              .reshape(T, CTOK, DOUT)
        full[:, c * CTOK:(c + 1) * CTOK, :] = oc.astype(np.float32)
    return full.reshape(T, B, S, DOUT)


# revision 24
# speedup vs baseline: 1.0063x; 1.0063x over previous
"""LoRA-linear Trainium2 Bass kernel (bf16 in/out, chunk-streamed, pipelined).

Computes, for T adapters: out[t] = x @ W.T + (x @ A_t.T) @ B_t.T + bias
Output: [T, B, S, Dout] float32 (stored bf16 on-device, widened on host).

Sharding: data-parallel over tokens across 8 NeuronCores (2048 tokens/core);
W/bias/selected-LoRA replicated. All matmul inputs are bf16 (host cast);
accumulation is fp32 in PSUM; the output is written to HBM as bf16 (16 MB
per core instead of 32), far inside the 2e-2 absmax-relative gate.

Lessons from the NTFF traces baked in here:
 * Every load/store is a single plain DMA with >=2KB contiguous
   per-partition runs (host pre-packs all layouts). Small or strided
   patterns cost 100s of descriptors; descriptor-gen serializes on the
   issuing engine's queue (~0.7us per 128-descriptor DMA) and tiny runs
   drain far below line rate. An early version spent 90us issuing 128
   per-(c,m,t) stores; this one issues 32 per-(c,m) stores of
   [128, T*512] (4KB/partition contiguous both sides).
 * Warm-up matmuls read a GpSimd-memset tile, not a DMA'd tile, so the
   HAM clock-gate ramp (needs ~3.4us of PE activity) runs during the DMA
   prologue instead of after it.
 * Loads are ordered by first use and sliced to stay just ahead of the
   consumer: x-chunk0 in two k-halves (phase1 k0-3 starts on the first),
   then W m-tile 0 alone, m1-3, then B/bias, W m4-7, x chunks 1-3.
 * DVE tensor_tensor with a PSUM operand runs at 1x; all-bf16 SBUF adds
   run at 2x. Per (c,m): deltas t0/t1 land in one 2-bank PSUM tile that
   is added in a single 1024-wide op (base broadcast via a 0-stride AP),
   t2/t3 land in another, which ScalarE copies to bf16 SBUF (one fused
   ACTIVATE) before a 1024-wide 2x DVE add. DVE (~61us) and ScalarE
   (~60us) both sit under the PE's ~77us.

Per-core schedule, chunk-major over 4 token-chunks of 512 tokens:
  phase1(c) lowT[32t+j, tok] = sum_d A_t[j,d] x[tok,d]  (8 k-matmuls)
  base(c,m) W[m-tile] @ x_c.T -> PSUM (8 k-matmuls, 216ns cadence);
            ScalarE evacuates with bias folded in, bf16
  delta     per t: 4 row-group matmuls at tile_position (32t,0),
            concurrent in the PE array (~0.4us for all four)
  add/store DVE adds write bf16 slices of od[128, T*512]; one 512KB store
Deltas for (c, m-1) are emitted after base (c, m) so the PE never waits
on DVE/ScalarE; phase1(c+1) is slotted mid-chunk, off the critical path.
"""

import sys

if "/opt/trn_rl_repo" not in sys.path:
    sys.path.insert(0, "/opt/trn_rl_repo")

from contextlib import ExitStack

import ml_dtypes
import numpy as np

import concourse.bacc as bacc
import concourse.bass as bass
import concourse.mybir as mybir
import concourse.tile as tile
from concourse import bass_utils

# Problem constants (hardcoded per spec).
B, S, DIN, DOUT, R, NL, T = 4, 4096, 1024, 1024, 16, 8, 4
NCORES = 8
NTOK = B * S                 # 16384
CTOK = NTOK // NCORES        # 2048 tokens per core
KT = DIN // 128              # 8 k-tiles
MT = DOUT // 128             # 8 dout-tiles
CH = 4                       # token chunks per core
CW = CTOK // CH              # 512 tokens per chunk

# Warm-up must keep the PE continuously busy from engine-start (~7.5us)
# until phase1's inputs land (~11.6us): any idle gap resets the HAM
# activity window and the main loop opens at 1.2 GHz instead of 2.4.
WARM1 = 46

F32 = mybir.dt.float32
BF16 = mybir.dt.bfloat16
NPBF16 = ml_dtypes.bfloat16


def _build_program():
    nc = bacc.Bacc("TRN2", target_bir_lowering=False, debug=False,
                   num_devices=NCORES)

    # All DRAM layouts are pre-packed on host so every DMA is a plain
    # contiguous [128, n] transfer.
    xc = nc.dram_tensor("xc", [CH, 128, KT * CW], BF16, kind="ExternalInput").ap()
    wt = nc.dram_tensor("wt", [128, MT * KT * 128], BF16, kind="ExternalInput").ap()
    atp = nc.dram_tensor("atp", [128, KT * 128], BF16, kind="ExternalInput").ap()
    btp = nc.dram_tensor("btp", [128, DOUT], BF16, kind="ExternalInput").ap()
    biasc = nc.dram_tensor("biasc", [128, MT], F32, kind="ExternalInput").ap()
    out = nc.dram_tensor("out", [CH, MT, 128, T * CW], BF16,
                         kind="ExternalOutput").ap()

    with tile.TileContext(nc) as tc, ExitStack() as ctx:
        const = ctx.enter_context(tc.tile_pool(name="const", bufs=1))
        lw_pool = ctx.enter_context(tc.tile_pool(name="lw", bufs=2))
        bsb_pool = ctx.enter_context(tc.tile_pool(name="bsb", bufs=3))
        ds_pool = ctx.enter_context(tc.tile_pool(name="ds", bufs=3))
        od_pool = ctx.enter_context(tc.tile_pool(name="od", bufs=6))
        bp_ps = ctx.enter_context(tc.tile_pool(name="bp_ps", bufs=2, space="PSUM"))
        ph_ps = ctx.enter_context(tc.tile_pool(name="ph_ps", bufs=2, space="PSUM"))
        dps_ps = ctx.enter_context(tc.tile_pool(name="dps_ps", bufs=1, space="PSUM"))
        dpd_ps = ctx.enter_context(tc.tile_pool(name="dpd_ps", bufs=1, space="PSUM"))

        # Warm-up source: memset by GpSimd (~6us mark), no DMA dependency.
        wsrc = const.tile([128, 128], BF16, tag="wsrc")
        nc.gpsimd.memset(wsrc[:], 0.0)

        # ALL loads ride the sync ring in consumption order, so the gating
        # transfers (A, x-chunk0) drain at the full SDMA rate instead of
        # sharing it with a second ring.
        at_all = const.tile([128, KT * 128], BF16, tag="at")
        bt_s = const.tile([128, DOUT], BF16, tag="bt")
        bias_s = const.tile([128, MT], F32, tag="bias")

        # Big loads on the sync ring, in consumption order: x chunk 0
        # (phase1 needs only x), W in two m-halves, then chunks 1-3.
        xc_t = []
        for c in range(CH):
            xc_t.append(const.tile([128, KT * CW], BF16, tag=f"xc{c}",
                                   name=f"xc{c}"))
        wt_all = const.tile([128, MT * KT * 128], BF16, tag="wt")

        # x chunk 0 in two k-halves (phase1 k0-3 can start on the first),
        # then W in m-slices sized to stay ahead of the base m-loop.
        hx = KT * CW // 2
        nc.sync.dma_start(at_all[:], atp[:, :])
        nc.sync.dma_start(xc_t[0][:, 0:hx], xc[0][:, 0:hx])
        nc.sync.dma_start(xc_t[0][:, hx:2 * hx], xc[0][:, hx:2 * hx])
        msz = KT * 128
        nc.sync.dma_start(wt_all[:, 0:msz], wt[:, 0:msz])
        nc.sync.dma_start(wt_all[:, msz:4 * msz], wt[:, msz:4 * msz])
        nc.sync.dma_start(bt_s[:], btp[:, :])
        nc.sync.dma_start(bias_s[:], biasc[:, :])
        nc.sync.dma_start(wt_all[:, 4 * msz:MT * msz], wt[:, 4 * msz:MT * msz])
        for c in range(1, CH):
            nc.sync.dma_start(xc_t[c][:], xc[c])

        lwt = {}

        def emit_phase1(c):
            ph = ph_ps.tile([128, CW], F32, tag="ph", name=f"ph{c}")
            for k in range(KT):
                nc.tensor.matmul(
                    ph[:],
                    at_all[:, bass.ts(k, 128)],
                    xc_t[c][:, bass.ts(k, CW)],
                    start=(k == 0), stop=(k == KT - 1),
                )
            t_ = lw_pool.tile([128, CW], BF16, tag="lw", name=f"lw{c}")
            nc.scalar.copy(t_[:], ph[:])
            lwt[c] = t_

        # Warm-up: the HAM clock gate needs ~3.4us of sustained PE activity
        # to unthrottle 1.2 -> 2.4 GHz; run it on the memset tile while the
        # input DMAs stream.
        warm = ph_ps.tile([128, CW], F32, tag="ph", name="warm")
        for _ in range(WARM1):
            nc.tensor.matmul(warm[:, 0:128], wsrc[:], wsrc[:],
                             start=True, stop=True)
        emit_phase1(0)

        def emit_base_mms(c, m):
            bp = bp_ps.tile([128, CW], F32, tag="bp", name=f"bp{c}_{m}")
            for k in range(KT):
                nc.tensor.matmul(
                    bp[:],
                    wt_all[:, m * (KT * 128) + k * 128:
                           m * (KT * 128) + (k + 1) * 128],
                    xc_t[c][:, bass.ts(k, CW)],
                    start=(k == 0), stop=(k == KT - 1),
                )
            return bp

        def emit_evac(c, m, bp):
            bsb = bsb_pool.tile([128, CW], BF16, tag="bsb", name=f"bsb{c}_{m}")
            nc.scalar.activation(
                bsb[:], bp[:],
                mybir.ActivationFunctionType.Identity,
                bias=bias_s[:, m:m + 1],
            )
            return bsb

        def emit_delta(c, m, bsb, last=False):
            d01 = dps_ps.tile([128, 2 * CW], F32, tag="dps", name=f"dp{c}_{m}_01")
            d23 = dpd_ps.tile([128, 2 * CW], F32, tag="dpd", name=f"dp{c}_{m}_23")
            outs = [d01[:, 0:CW], d01[:, CW:2 * CW],
                    d23[:, 0:CW], d23[:, CW:2 * CW]]
            for t in range(T):
                nc.tensor.matmul(
                    outs[t],
                    bt_s[32 * t:32 * t + R, bass.ts(m, 128)],
                    lwt[c][32 * t:32 * t + R, :],
                    start=True, stop=True,
                    tile_position=(32 * t, 0),
                )
            # Fused ScalarE evacuation of t2/t3 ahead of the base evac in the
            # ACT queue so the (single-buffered) d23 bank frees early.
            ds = ds_pool.tile([128, 2 * CW], BF16, tag="ds", name=f"ds{c}_{m}")
            nc.scalar.copy(ds[:], d23[:])
            od = od_pool.tile([128, T * CW], BF16, tag="od", name=f"od{c}_{m}")
            # Fused pair-adds: base broadcast along a 0-stride outer dim.
            bsb2 = bsb[:].rearrange("p (o w) -> p o w", o=1).broadcast_to(
                [128, 2, CW])
            nc.vector.tensor_add(
                od[:, 0:2 * CW].rearrange("p (o w) -> p o w", o=2),
                bsb2, d01[:].rearrange("p (o w) -> p o w", o=2))
            if last:
                # Final tile: two half-stores on different HWDGE rings so
                # drain + HBM write receipt overlap at the very end.
                nc.scalar.dma_start(out[c, m, :, 0:2 * CW], od[:, 0:2 * CW])
            nc.vector.tensor_add(
                od[:, 2 * CW:4 * CW].rearrange("p (o w) -> p o w", o=2),
                bsb2, ds[:].rearrange("p (o w) -> p o w", o=2))
            if last:
                nc.sync.dma_start(out[c, m, :, 2 * CW:4 * CW],
                                  od[:, 2 * CW:4 * CW])
            else:
                nc.sync.dma_start(out[c, m, :, :], od[:])

        prev = None
        for c in range(CH):
            for m in range(MT):
                if (c, m) == (CH - 1, MT - 1) and prev is not None:
                    # Break the software pipeline for the final tile: its
                    # predecessor's deltas/adds run concurrently with this
                    # base (deltas have no base dependency), so the tail
                    # chain after the last matmul shortens by ~1.5us.
                    emit_delta(*prev)
                    prev = None
                bp = emit_base_mms(c, m)
                if m == 4 and c + 1 < CH:
                    emit_phase1(c + 1)
                if prev is not None:
                    emit_delta(*prev)
                bsb = emit_evac(c, m, bp)
                prev = (c, m, bsb)
        emit_delta(*prev, last=True)

    nc.compile()
    return nc


_NC = None


def _get_program():
    global _NC
    if _NC is None:
        _NC = _build_program()
    return _NC


def kernel(**inputs):
    x = np.ascontiguousarray(np.asarray(inputs["x"], dtype=np.float32))
    W = np.asarray(inputs["W"], dtype=np.float32)
    bias_v = np.asarray(inputs["bias"], dtype=np.float32)
    lora_A = np.asarray(inputs["lora_A"], dtype=np.float32)
    lora_B = np.asarray(inputs["lora_B"], dtype=np.float32)
    tuner_index = np.asarray(inputs["tuner_index"]).astype(np.int64)

    assert x.shape == (B, S, DIN) and W.shape == (DOUT, DIN)
    assert tuner_index.shape == (T,)

    A_sel = lora_A[tuner_index]          # [T, R, Din]
    B_sel = lora_B[tuner_index]          # [T, Dout, R]

    toks = x.reshape(NTOK, DIN)
    # wt[p, m, k, n] = W[m*128+n, k*128+p]
    wt = np.ascontiguousarray(
        W.reshape(MT, 128, KT, 128).transpose(3, 0, 2, 1)
    ).astype(NPBF16).reshape(128, MT * KT * 128)
    # atp_flat[d, 32t+j] = A_sel[t, j, d]; then [p, k, j] = [k*128+p, j]
    atp_flat = np.zeros((DIN, 128), np.float32)
    atp_flat.reshape(DIN, T, 32)[:, :, :R] = A_sel.transpose(2, 0, 1)
    atp = np.ascontiguousarray(
        atp_flat.reshape(KT, 128, 128).transpose(1, 0, 2)
    ).astype(NPBF16).reshape(128, KT * 128)
    btp = np.zeros((128, DOUT), NPBF16)
    btp.reshape(T, 32, DOUT)[:, :R, :] = B_sel.transpose(0, 2, 1).astype(NPBF16)
    biasc = np.ascontiguousarray(bias_v.reshape(MT, 128).T)   # [128, MT]

    in_maps = []
    for c in range(NCORES):
        xcore = toks[c * CTOK:(c + 1) * CTOK]            # [2048, 1024]
        # xh[ch, p, k, w] = x[ch*512+w, k*128+p]
        xch = np.ascontiguousarray(
            xcore.reshape(CH, CW, KT, 128).transpose(0, 3, 2, 1)
        ).astype(NPBF16).reshape(CH, 128, KT * CW)
        in_maps.append({
            "xc": xch,
            "wt": wt,
            "atp": atp,
            "btp": btp,
            "biasc": biasc,
        })

    nc = _get_program()
    res = bass_utils.run_bass_kernel_spmd(nc, in_maps, core_ids=list(range(NCORES)))

    full = np.empty((T, NTOK, DOUT), np.float32)
    for c in range(NCORES):
        o = np.asarray(res.results[c]["out"])   # [CH, MT, 128, T*CW] bf16
        # o[ch, m, p, t, w] -> [t, ch*CW + w, m*128 + p]
        oc = o.reshape(CH, MT, 128, T, CW).transpose(3, 0, 4, 1, 2) \
              .reshape(T, CTOK, DOUT)
        full[:, c * CTOK:(c + 1) * CTOK, :] = oc.astype(np.float32)
    return full.reshape(T, B, S, DOUT)


# revision 27
# speedup vs baseline: 1.0170x; 1.0107x over previous
"""LoRA-linear Trainium2 Bass kernel (bf16 in/out, chunk-streamed, pipelined).

Computes, for T adapters: out[t] = x @ W.T + (x @ A_t.T) @ B_t.T + bias
Output: [T, B, S, Dout] float32 (stored bf16 on-device, widened on host).

Sharding: data-parallel over tokens across 8 NeuronCores (2048 tokens/core);
W/bias/selected-LoRA replicated. All matmul inputs are bf16 (host cast);
accumulation is fp32 in PSUM; the output is written to HBM as bf16 (16 MB
per core instead of 32), far inside the 2e-2 absmax-relative gate.

Lessons from the NTFF traces baked in here:
 * Every load/store is a single plain DMA with >=2KB contiguous
   per-partition runs (host pre-packs all layouts). Small or strided
   patterns cost 100s of descriptors; descriptor-gen serializes on the
   issuing engine's queue (~0.7us per 128-descriptor DMA) and tiny runs
   drain far below line rate. An early version spent 90us issuing 128
   per-(c,m,t) stores; this one issues 32 per-(c,m) stores of
   [128, T*512] (4KB/partition contiguous both sides).
 * Warm-up matmuls read a GpSimd-memset tile, not a DMA'd tile, so the
   HAM clock-gate ramp (needs ~3.4us of PE activity) runs during the DMA
   prologue instead of after it.
 * Loads are ordered by first use and sliced to stay just ahead of the
   consumer: x-chunk0 in two k-halves (phase1 k0-3 starts on the first),
   then W m-tile 0 alone, m1-3, then B/bias, W m4-7, x chunks 1-3.
 * DVE tensor_tensor with a PSUM operand runs at 1x; all-bf16 SBUF adds
   run at 2x. Per (c,m): deltas t0/t1 land in one 2-bank PSUM tile that
   is added in a single 1024-wide op (base broadcast via a 0-stride AP),
   t2/t3 land in another, which ScalarE copies to bf16 SBUF (one fused
   ACTIVATE) before a 1024-wide 2x DVE add. DVE (~61us) and ScalarE
   (~60us) both sit under the PE's ~77us.

Per-core schedule, chunk-major over 4 token-chunks of 512 tokens:
  phase1(c) lowT[32t+j, tok] = sum_d A_t[j,d] x[tok,d]  (8 k-matmuls)
  base(c,m) W[m-tile] @ x_c.T -> PSUM (8 k-matmuls, 216ns cadence);
            ScalarE evacuates with bias folded in, bf16
  delta     per t: 4 row-group matmuls at tile_position (32t,0),
            concurrent in the PE array (~0.4us for all four)
  add/store DVE adds write bf16 slices of od[128, T*512]; one 512KB store
Deltas for (c, m-1) are emitted after base (c, m) so the PE never waits
on DVE/ScalarE; phase1(c+1) is slotted mid-chunk, off the critical path.
"""

import sys

if "/opt/trn_rl_repo" not in sys.path:
    sys.path.insert(0, "/opt/trn_rl_repo")

from contextlib import ExitStack

import ml_dtypes
import numpy as np

import concourse.bacc as bacc
import concourse.bass as bass
import concourse.mybir as mybir
import concourse.tile as tile
from concourse import bass_utils

# Problem constants (hardcoded per spec).
B, S, DIN, DOUT, R, NL, T = 4, 4096, 1024, 1024, 16, 8, 4
NCORES = 8
NTOK = B * S                 # 16384
CTOK = NTOK // NCORES        # 2048 tokens per core
KT = DIN // 128              # 8 k-tiles
MT = DOUT // 128             # 8 dout-tiles
CH = 4                       # token chunks per core
CW = CTOK // CH              # 512 tokens per chunk

# Warm-up must keep the PE continuously busy from engine-start (~7.5us)
# until phase1's inputs land (~11.6us): any idle gap resets the HAM
# activity window and the main loop opens at 1.2 GHz instead of 2.4.
WARM1 = 46

F32 = mybir.dt.float32
BF16 = mybir.dt.bfloat16
NPBF16 = ml_dtypes.bfloat16


def _build_program():
    nc = bacc.Bacc("TRN2", target_bir_lowering=False, debug=False,
                   num_devices=NCORES)

    # All DRAM layouts are pre-packed on host so every DMA is a plain
    # contiguous [128, n] transfer.
    xc = nc.dram_tensor("xc", [CH, 128, KT * CW], BF16, kind="ExternalInput").ap()
    wt = nc.dram_tensor("wt", [128, MT * KT * 128], BF16, kind="ExternalInput").ap()
    atp = nc.dram_tensor("atp", [128, KT * 128], BF16, kind="ExternalInput").ap()
    btp = nc.dram_tensor("btp", [128, DOUT], BF16, kind="ExternalInput").ap()
    biasc = nc.dram_tensor("biasc", [128, MT], F32, kind="ExternalInput").ap()
    out = nc.dram_tensor("out", [CH, MT, 128, T * CW], BF16,
                         kind="ExternalOutput").ap()

    with tile.TileContext(nc) as tc, ExitStack() as ctx:
        const = ctx.enter_context(tc.tile_pool(name="const", bufs=1))
        lw_pool = ctx.enter_context(tc.tile_pool(name="lw", bufs=2))
        bsb_pool = ctx.enter_context(tc.tile_pool(name="bsb", bufs=3))
        ds_pool = ctx.enter_context(tc.tile_pool(name="ds", bufs=3))
        od_pool = ctx.enter_context(tc.tile_pool(name="od", bufs=6))
        bp_ps = ctx.enter_context(tc.tile_pool(name="bp_ps", bufs=2, space="PSUM"))
        ph_ps = ctx.enter_context(tc.tile_pool(name="ph_ps", bufs=2, space="PSUM"))
        dps_ps = ctx.enter_context(tc.tile_pool(name="dps_ps", bufs=1, space="PSUM"))
        dpd_ps = ctx.enter_context(tc.tile_pool(name="dpd_ps", bufs=1, space="PSUM"))

        # Warm-up source: memset by GpSimd (~6us mark), no DMA dependency.
        wsrc = const.tile([128, 128], BF16, tag="wsrc")
        nc.gpsimd.memset(wsrc[:], 0.0)

        # ALL loads ride the sync ring in consumption order, so the gating
        # transfers (A, x-chunk0) drain at the full SDMA rate instead of
        # sharing it with a second ring.
        at_all = const.tile([128, KT * 128], BF16, tag="at")
        bt_s = const.tile([128, DOUT], BF16, tag="bt")
        bias_s = const.tile([128, MT], F32, tag="bias")

        # Big loads on the sync ring, in consumption order: x chunk 0
        # (phase1 needs only x), W in two m-halves, then chunks 1-3.
        xc_t = []
        for c in range(CH):
            xc_t.append(const.tile([128, KT * CW], BF16, tag=f"xc{c}",
                                   name=f"xc{c}"))
        wt_all = const.tile([128, MT * KT * 128], BF16, tag="wt")

        # x chunk 0 in two k-halves (phase1 k0-3 can start on the first),
        # then W in m-slices sized to stay ahead of the base m-loop.
        hx = KT * CW // 2
        msz = KT * 128
        nc.sync.dma_start(at_all[:], atp[:, :])
        nc.sync.dma_start(xc_t[0][:, 0:hx], xc[0][:, 0:hx])
        # W m-tile 0 ahead of x-chunk0's second half: base(0,0) k0-3 can
        # then start on the first half and stream into k4-7 as it lands.
        nc.sync.dma_start(wt_all[:, 0:msz], wt[:, 0:msz])
        nc.sync.dma_start(xc_t[0][:, hx:2 * hx], xc[0][:, hx:2 * hx])
        nc.sync.dma_start(wt_all[:, msz:4 * msz], wt[:, msz:4 * msz])
        nc.sync.dma_start(bt_s[:], btp[:, :])
        nc.sync.dma_start(bias_s[:], biasc[:, :])
        nc.sync.dma_start(wt_all[:, 4 * msz:MT * msz], wt[:, 4 * msz:MT * msz])
        for c in range(1, CH):
            nc.sync.dma_start(xc_t[c][:], xc[c])

        lwt = {}

        def emit_phase1(c):
            ph = ph_ps.tile([128, CW], F32, tag="ph", name=f"ph{c}")
            for k in range(KT):
                nc.tensor.matmul(
                    ph[:],
                    at_all[:, bass.ts(k, 128)],
                    xc_t[c][:, bass.ts(k, CW)],
                    start=(k == 0), stop=(k == KT - 1),
                )
            t_ = lw_pool.tile([128, CW], BF16, tag="lw", name=f"lw{c}")
            nc.scalar.copy(t_[:], ph[:])
            lwt[c] = t_

        # Warm-up: the HAM clock gate needs ~3.4us of sustained PE activity
        # to unthrottle 1.2 -> 2.4 GHz; run it on the memset tile while the
        # input DMAs stream.
        warm = ph_ps.tile([128, CW], F32, tag="ph", name="warm")
        for _ in range(WARM1):
            nc.tensor.matmul(warm[:, 0:128], wsrc[:], wsrc[:],
                             start=True, stop=True)

        def emit_base_mms(c, m):
            bp = bp_ps.tile([128, CW], F32, tag="bp", name=f"bp{c}_{m}")
            for k in range(KT):
                nc.tensor.matmul(
                    bp[:],
                    wt_all[:, m * (KT * 128) + k * 128:
                           m * (KT * 128) + (k + 1) * 128],
                    xc_t[c][:, bass.ts(k, CW)],
                    start=(k == 0), stop=(k == KT - 1),
                )
            return bp

        def emit_evac(c, m, bp):
            bsb = bsb_pool.tile([128, CW], BF16, tag="bsb", name=f"bsb{c}_{m}")
            nc.scalar.activation(
                bsb[:], bp[:],
                mybir.ActivationFunctionType.Identity,
                bias=bias_s[:, m:m + 1],
            )
            return bsb

        def emit_delta(c, m, bsb, last=False):
            d01 = dps_ps.tile([128, 2 * CW], F32, tag="dps", name=f"dp{c}_{m}_01")
            d23 = dpd_ps.tile([128, 2 * CW], F32, tag="dpd", name=f"dp{c}_{m}_23")
            outs = [d01[:, 0:CW], d01[:, CW:2 * CW],
                    d23[:, 0:CW], d23[:, CW:2 * CW]]
            for t in range(T):
                nc.tensor.matmul(
                    outs[t],
                    bt_s[32 * t:32 * t + R, bass.ts(m, 128)],
                    lwt[c][32 * t:32 * t + R, :],
                    start=True, stop=True,
                    tile_position=(32 * t, 0),
                )
            # Fused ScalarE evacuation of t2/t3 ahead of the base evac in the
            # ACT queue so the (single-buffered) d23 bank frees early.
            ds = ds_pool.tile([128, 2 * CW], BF16, tag="ds", name=f"ds{c}_{m}")
            nc.scalar.copy(ds[:], d23[:])
            od = od_pool.tile([128, T * CW], BF16, tag="od", name=f"od{c}_{m}")
            # Fused pair-adds: base broadcast along a 0-stride outer dim.
            bsb2 = bsb[:].rearrange("p (o w) -> p o w", o=1).broadcast_to(
                [128, 2, CW])
            nc.vector.tensor_add(
                od[:, 0:2 * CW].rearrange("p (o w) -> p o w", o=2),
                bsb2, d01[:].rearrange("p (o w) -> p o w", o=2))
            if last:
                # Final tile: two half-stores on different HWDGE rings so
                # drain + HBM write receipt overlap at the very end.
                nc.scalar.dma_start(out[c, m, :, 0:2 * CW], od[:, 0:2 * CW])
            nc.vector.tensor_add(
                od[:, 2 * CW:4 * CW].rearrange("p (o w) -> p o w", o=2),
                bsb2, ds[:].rearrange("p (o w) -> p o w", o=2))
            if last:
                nc.sync.dma_start(out[c, m, :, 2 * CW:4 * CW],
                                  od[:, 2 * CW:4 * CW])
            else:
                nc.sync.dma_start(out[c, m, :, :], od[:])

        prev = None
        for c in range(CH):
            for m in range(MT):
                if (c, m) == (CH - 1, MT - 1) and prev is not None:
                    # Break the software pipeline for the final tile: its
                    # predecessor's deltas/adds run concurrently with this
                    # base (deltas have no base dependency), so the tail
                    # chain after the last matmul shortens by ~1.5us.
                    emit_delta(*prev)
                    prev = None
                bp = emit_base_mms(c, m)
                if (c, m) == (0, 0):
                    # phase1(c0) after the first base tile: base(0,0) k0-3
                    # is gated only by xc0h1+wt_m0, so the PE saturates
                    # earlier than phase1 (which also needs at_all) would.
                    emit_phase1(0)
                if m == 4 and c + 1 < CH:
                    emit_phase1(c + 1)
                if prev is not None:
                    emit_delta(*prev)
                bsb = emit_evac(c, m, bp)
                prev = (c, m, bsb)
        emit_delta(*prev, last=True)

    nc.compile()
    return nc


_NC = None


def _get_program():
    global _NC
    if _NC is None:
        _NC = _build_program()
    return _NC


def kernel(**inputs):
    x = np.ascontiguousarray(np.asarray(inputs["x"], dtype=np.float32))
    W = np.asarray(inputs["W"], dtype=np.float32)
    bias_v = np.asarray(inputs["bias"], dtype=np.float32)
    lora_A = np.asarray(inputs["lora_A"], dtype=np.float32)
    lora_B = np.asarray(inputs["lora_B"], dtype=np.float32)
    tuner_index = np.asarray(inputs["tuner_index"]).astype(np.int64)

    assert x.shape == (B, S, DIN) and W.shape == (DOUT, DIN)
    assert tuner_index.shape == (T,)

    A_sel = lora_A[tuner_index]          # [T, R, Din]
    B_sel = lora_B[tuner_index]          # [T, Dout, R]

    toks = x.reshape(NTOK, DIN)
    # wt[p, m, k, n] = W[m*128+n, k*128+p]
    wt = np.ascontiguousarray(
        W.reshape(MT, 128, KT, 128).transpose(3, 0, 2, 1)
    ).astype(NPBF16).reshape(128, MT * KT * 128)
    # atp_flat[d, 32t+j] = A_sel[t, j, d]; then [p, k, j] = [k*128+p, j]
    atp_flat = np.zeros((DIN, 128), np.float32)
    atp_flat.reshape(DIN, T, 32)[:, :, :R] = A_sel.transpose(2, 0, 1)
    atp = np.ascontiguousarray(
        atp_flat.reshape(KT, 128, 128).transpose(1, 0, 2)
    ).astype(NPBF16).reshape(128, KT * 128)
    btp = np.zeros((128, DOUT), NPBF16)
    btp.reshape(T, 32, DOUT)[:, :R, :] = B_sel.transpose(0, 2, 1).astype(NPBF16)
    biasc = np.ascontiguousarray(bias_v.reshape(MT, 128).T)   # [128, MT]

    in_maps = []
    for c in range(NCORES):
        xcore = toks[c * CTOK:(c + 1) * CTOK]            # [2048, 1024]
        # xh[ch, p, k, w] = x[ch*512+w, k*128+p]
        xch = np.ascontiguousarray(
            xcore.reshape(CH, CW, KT, 128).transpose(0, 3, 2, 1)
        ).astype(NPBF16).reshape(CH, 128, KT * CW)
        in_maps.append({
            "xc": xch,
            "wt": wt,
            "atp": atp,
            "btp": btp,
            "biasc": biasc,
        })

    nc = _get_program()
    res = bass_utils.run_bass_kernel_spmd(nc, in_maps, core_ids=list(range(NCORES)))

    full = np.empty((T, NTOK, DOUT), np.float32)
    for c in range(NCORES):
        o = np.asarray(res.results[c]["out"])   # [CH, MT, 128, T*CW] bf16
        # o[ch, m, p, t, w] -> [t, ch*CW + w, m*128 + p]
        oc = o.reshape(CH, MT, 128, T, CW).transpose(3, 0, 4, 1, 2) \
              .reshape(T, CTOK, DOUT)
        full[:, c * CTOK:(c + 1) * CTOK, :] = oc.astype(np.float32)
    return full.reshape(T, B, S, DOUT)
